# revision 38
# baseline (speedup 1.0000x reference)
"""Trainium2 Bass kernel for entity-attention input scaling.

Computes, per batch row b:
    A_k = wd[b] @ e_k[b]          (k = 1, 2)   [S]
    alpha_k = softmax(A_k)
    out[b]  = wM[b] * 0.5 * (alpha_1^2 + alpha_2^2)[:, None]

Sharding: pure data parallel over the batch dim. B=32 batches are split
4-per-core over 8 NeuronCores; no cross-core communication.

Per-core pipeline (per local batch), memory-roofline bound (~50MB DMA/core):
  - wd streamed in 2MB contiguous chunks -> SBUF [128, 4096]
    (s = 2048*c + 16*p + j; p = partition, j in 0..15)
  - logits on the DVE: one fused scalar_tensor_tensor (product + free-axis
    accumulate) per [128, 256] tile against host-broadcast e_k
    -> psA[128, 64] (A_k per (tile, k) col).
  - softmax stats: row max via DVE reduce + PE transpose + DVE reduce;
    exp on ACT with per-partition accumulate, cross-partition sums via a
    ones-vector matmul, 1/Z on DVE reciprocal.  alpha is assembled as
    c_1*E_1^2 + c_2*E_2^2 with c_k = 0.5/Z_k^2 broadcast across partitions
    by a rank-1 matmul (no Ln -> single ACT table load).
  - out = wM * alpha via per-partition scaled multiply, split ACT/DVE.
  - The per-batch stats chain is a long cross-engine dependency chain, so
    emission is software-pipelined at op granularity: batch b's stats and
    finals closures are emitted one per j-tile inside batch b+2's phase 1,
    and the two trailing batches' chains are interleaved with each other,
    so the DVE keeps streaming dot products while stats hop across engines.

Measured on 8 axon-tunneled TRN2 cores: 145.3us NEFF exec, rel err 4.0e-06
(memory roofline for the 50.4MB/core of HBM traffic is ~127-145us).
"""

import numpy as np
from contextlib import ExitStack

import concourse.bacc as bacc
import concourse.tile as tile
from concourse import mybir
from concourse.bass_utils import run_bass_kernel_spmd

B, S, D = 32, 4096, 256
N_CORES = 8
BPC = B // N_CORES          # batches per core
CHUNK = 2048                # S-rows per DMA chunk (2MB)
NCH = S // CHUNK            # chunks per batch
JP = CHUNK // 128           # 128-row tiles per chunk
NT = S // 128               # 128-row tiles per batch
F32 = mybir.dt.float32
AF = mybir.ActivationFunctionType
ALU = mybir.AluOpType
CORE_IDS = list(range(N_CORES))

_cache: dict = {}


def _build():
    nc = bacc.Bacc("TRN2", target_bir_lowering=False, debug=False,
                   num_devices=N_CORES)
    wd_h = nc.declare_dram_parameter("wd", [BPC, S, D], F32, isOutput=False)
    wM_h = nc.declare_dram_parameter("wM", [BPC, S, D], F32, isOutput=False)
    # erow[0, ((b*2 + k)*D + d)] = e_k[b, d]; broadcast on-chip (8KB DMA
    # instead of a 1MB pre-broadcast copy)
    er_h = nc.declare_dram_parameter("erow", [1, BPC * 2 * D], F32,
                                     isOutput=False)
    id_h = nc.declare_dram_parameter("ident", [128, 128], F32, isOutput=False)
    out_h = nc.declare_dram_parameter("out", [BPC, S, D], F32, isOutput=True)

    def chunk_view(h, b, c):
        # [CHUNK, D] contiguous rows -> [128, JP*D]; s = CHUNK*c + JP*p + j
        return h[b, CHUNK * c:CHUNK * (c + 1), :].rearrange(
            "(p j) d -> p (j d)", p=128)

    with tile.TileContext(nc) as tc, ExitStack() as ctx:
        consts = ctx.enter_context(tc.tile_pool(name="consts", bufs=1))
        wd_pool = ctx.enter_context(tc.tile_pool(name="wdp", bufs=3))
        wm_pool = ctx.enter_context(tc.tile_pool(name="wmp", bufs=4))
        out_pool = ctx.enter_context(tc.tile_pool(name="outp", bufs=3))
        scr_pool = ctx.enter_context(tc.tile_pool(name="scrp", bufs=2))
        sm_pool = ctx.enter_context(tc.tile_pool(name="smalls", bufs=2))
        al_pool = ctx.enter_context(tc.tile_pool(name="alphas", bufs=2))
        # two stats chains can be in flight at the kernel tail
        pss_pool = ctx.enter_context(tc.tile_pool(name="pss", bufs=4, space="PSUM"))
        psb_pool = ctx.enter_context(tc.tile_pool(name="psb", bufs=2, space="PSUM"))

        # constants: memset where possible, tiny DMAs on the store ring
        # (idle at kernel start) so nothing delays the first wd chunks.
        # memsets run on the DVE: gpsimd boots slowest (~5-8us preamble) and
        # the onesrow memset heads the e-broadcast chain that gates the
        # first dot product.
        onescol = consts.tile([128, 1], F32)
        nc.vector.memset(onescol[:], 1.0)
        onesrow = consts.tile([1, 128], F32)
        nc.vector.memset(onesrow[:], 1.0)
        negone = consts.tile([1, 128], F32)
        nc.vector.memset(negone[:], -1.0)
        ident = consts.tile([128, 128], F32)
        nc.scalar.dma_start(ident[:], id_h[:])
        # e rows: 8KB DMA, then rank-1 matmul broadcast to all partitions
        erow = consts.tile([1, BPC * 2 * D], F32)
        nc.scalar.dma_start(erow[:], er_h[:])
        ebc = consts.tile([128, BPC * 2 * D], F32)
        for q in range(BPC * 2 * D // 512):
            qsl = slice(q * 512, (q + 1) * 512)
            eb_ps = psb_pool.tile([128, 512], F32, tag="ebps")
            nc.tensor.matmul(eb_ps[:], onesrow[:], erow[:, qsl],
                             start=True, stop=True)
            nc.scalar.copy(ebc[:, qsl], eb_ps[:])

        psAs = {}

        def phase1(b, interleave=None):
            # logits: psA[p, 2t+k] = sum_d wd[s(p,t), d] * e_k[d]
            # `interleave`: list of closures (previous batch's stats/finals)
            # emitted one per j-tile so the DVE program keeps streaming dot
            # products while the stats chain hops across engines.
            psA = al_pool.tile([128, 2 * NT], F32, tag="psA")
            psAs[b] = psA
            for c in range(NCH):
                # First chunk of the kernel arrives in 256KB eighths so the
                # DVE starts on first-touch data; steady state uses one 2MB
                # DMA.
                nparts = 8 if (b == 0 and c == 0) else 1
                jpp = JP // nparts
                wd_ch = wd_pool.tile([128, JP * D], F32, tag="wd")
                full = chunk_view(wd_h, b, c)
                for p_ in range(nparts):
                    fsl = slice(p_ * jpp * D, (p_ + 1) * jpp * D)
                    nc.sync.dma_start(wd_ch[:, fsl], full[:, fsl])
                for j in range(JP):
                    t = c * JP + j
                    wsl = wd_ch[:, j * D:(j + 1) * D]
                    for k in range(2):
                        scr = scr_pool.tile([128, D], F32, tag="scr")
                        nc.vector.scalar_tensor_tensor(
                            scr[:], wsl, 1.0,
                            ebc[:, (b * 2 + k) * D:(b * 2 + k + 1) * D],
                            op0=ALU.mult, op1=ALU.mult,
                            accum_out=psA[:, 2 * t + k:2 * t + k + 1])
                    if interleave:
                        interleave.pop(0)()
            while interleave:
                interleave.pop(0)()

        def build_phase23_ops(b):
            """Batch b's softmax + finals as a list of closures, emitted one
            per j-tile inside the next batch's phase 1 (or directly)."""
            psA = psAs.pop(b)
            st: dict = {}
            ops = []

            def op_mx():
                st["mx"] = sm_pool.tile([128, 1], F32, tag="mx", name="mx")
                nc.vector.tensor_reduce(st["mx"][:], psA[:],
                                        axis=mybir.AxisListType.X, op=ALU.max)

            def op_tmax():
                st["tmax"] = pss_pool.tile([1, 128], F32, tag="pssm", name="tmax")
                nc.tensor.transpose(st["tmax"][:], st["mx"][:], ident[:])

            def op_m2():
                st["m2"] = sm_pool.tile([1, 1], F32, tag="m2", name="m2")
                nc.vector.tensor_reduce(st["m2"][:], st["tmax"][:],
                                        axis=mybir.AxisListType.X, op=ALU.max)

            def op_mneg_mm():
                st["mneg_ps"] = pss_pool.tile([128, 1], F32, tag="pssm", name="mneg_ps")
                nc.tensor.matmul(st["mneg_ps"][:], negone[:], st["m2"][:],
                                 start=True, stop=True)

            def op_mneg_cp():
                st["mneg"] = sm_pool.tile([128, 1], F32, tag="mneg", name="mneg")
                nc.scalar.copy(st["mneg"][:], st["mneg_ps"][:])

            def op_exp(k):
                psA_v = psA[:].rearrange("p (t k) -> p k t", k=2)
                if "E" not in st:
                    st["E"] = al_pool.tile([128, 2 * NT], F32, tag="E", name="E")
                    st["s12"] = sm_pool.tile([128, 2], F32, tag="s12", name="s12")
                E_v = st["E"][:].rearrange("p (t k) -> p k t", k=2)
                nc.scalar.activation(E_v[:, k, :], psA_v[:, k, :], AF.Exp,
                                     bias=st["mneg"][:], scale=1.0,
                                     accum_out=st["s12"][:, k:k + 1])

            def op_zsum():
                st["zsum"] = pss_pool.tile([1, 2], F32, tag="pssm", name="zsum")
                nc.tensor.matmul(st["zsum"][:], onescol[:], st["s12"][:],
                                 start=True, stop=True)

            def op_zinv():
                st["zinv"] = sm_pool.tile([1, 2], F32, tag="zinv", name="zinv")
                nc.vector.reciprocal(st["zinv"][:], st["zsum"][:])
                st["zz"] = sm_pool.tile([1, 2], F32, tag="zz", name="zz")
                nc.vector.tensor_scalar(st["zz"][:], st["zinv"][:], 0.5, None,
                                        op0=ALU.mult)
                nc.vector.tensor_mul(st["zz"][:], st["zz"][:], st["zinv"][:])

            def op_cps():
                st["c_ps"] = pss_pool.tile([128, 2], F32, tag="pssm", name="c_ps")
                nc.tensor.matmul(st["c_ps"][:], onesrow[:], st["zz"][:],
                                 start=True, stop=True)

            def op_c12():
                st["c12"] = sm_pool.tile([128, 2], F32, tag="c12", name="c12")
                nc.scalar.copy(st["c12"][:], st["c_ps"][:])

            def op_esq():
                st["esq"] = al_pool.tile([128, 2 * NT], F32, tag="esq", name="esq")
                nc.vector.tensor_mul(st["esq"][:], st["E"][:], st["E"][:])

            def op_alpha():
                esq_v = st["esq"][:].rearrange("p (t k) -> p k t", k=2)
                atmp = al_pool.tile([128, NT], F32, tag="atmp")
                nc.vector.tensor_scalar_mul(atmp[:], esq_v[:, 1, :],
                                            st["c12"][:, 1:2])
                st["alpha"] = al_pool.tile([128, NT], F32, tag="alpha", name="alpha")
                nc.vector.scalar_tensor_tensor(st["alpha"][:], esq_v[:, 0, :],
                                               st["c12"][:, 0:1], atmp[:],
                                               op0=ALU.mult, op1=ALU.add)

            ops += [op_mx, op_tmax, op_m2, op_mneg_mm, op_mneg_cp,
                    lambda: op_exp(0), lambda: op_exp(1),
                    op_zsum, op_zinv, op_cps, op_c12, op_esq, op_alpha]

            # ---- out = wM * alpha ----
            # Last batch: 1MB quarters + all finals on the DVE so loads,
            # finals and stores pipeline tightly at the kernel tail.
            nparts = 4 if b == BPC - 1 else 1
            jpp = JP // nparts
            tail_ops = []

            def fin_part(c, p_, jpp_, eng, load):
                # one wM sub-load (optional) + its finals + its 1MB+ store
                def op(c=c, p_=p_, jpp_=jpp_, eng=eng, load=load):
                    wm_ch = st[("wm", c)]
                    out_ch = st[("out", c)]
                    fsl = slice(p_ * jpp_ * D, (p_ + 1) * jpp_ * D)
                    if load:
                        nc.sync.dma_start(wm_ch[:, fsl],
                                          chunk_view(wM_h, b, c)[:, fsl])
                    for j in range(p_ * jpp_, (p_ + 1) * jpp_):
                        t = c * JP + j
                        sl = slice(j * D, (j + 1) * D)
                        if eng is nc.vector:
                            nc.vector.tensor_scalar_mul(
                                out_ch[:, sl], wm_ch[:, sl],
                                st["alpha"][:, t:t + 1])
                        else:
                            nc.scalar.mul(out_ch[:, sl], wm_ch[:, sl],
                                          st["alpha"][:, t:t + 1])
                    nc.scalar.dma_start(
                        chunk_view(out_h, b, c)[:, fsl], out_ch[:, fsl])
                return op

            for c in range(NCH):
                def op_wm_alloc(b=b, c=c):
                    st[("wm", c)] = wm_pool.tile([128, JP * D], F32, tag="wm", name="wm")
                    st[("out", c)] = out_pool.tile([128, JP * D], F32, tag="out", name="out")
                ops.append(op_wm_alloc)
                if b == BPC - 2:
                    # Second-to-last batch: first half of each chunk on ACT
                    # during the dot stream (ACT has slack there); second
                    # half deferred to the DVE *after* the stream ends (via
                    # the tail pad below), so ACT's in-order queue is clear
                    # when the last batch's stats hops (exp is ACT-only)
                    # arrive.  Each half stores its own contiguous 1MB as
                    # soon as it finishes.
                    def op_wm_load2(b=b, c=c):
                        nc.sync.dma_start(st[("wm", c)][:],
                                          chunk_view(wM_h, b, c)[:])
                    ops.append(op_wm_load2)
                    ops.append(fin_part(c, 0, JP // 2, nc.scalar, False))
                    tail_ops.append(fin_part(c, 1, JP // 2, nc.vector, False))
                elif b == BPC - 1:
                    # Last batch: 1MB quarters, finals on the then-idle DVE.
                    for p_ in range(nparts):
                        ops.append(fin_part(c, p_, jpp, nc.vector, True))
                else:
                    ops.append(fin_part(c, 0, JP, nc.scalar, True))
            if tail_ops:
                # pad so the deferred closures pop only in phase1's trailing
                # while-loop, i.e. after every dot product is emitted
                # (phase1 pops one closure per j-tile; NCH*JP slots).
                ops += [lambda: None] * max(0, NCH * JP - len(ops))
                ops += tail_ops
            return ops

        # software pipeline: batch b's stats/finals closures are emitted one
        # per j-tile inside batch b+1's phase 1, so batches 0..2 fully drain
        # (stats on their engines, finals on ACT, stores) while the DVE
        # streams dot products; only batch 3's chain remains in the tail,
        # where it gets the then-idle DVE for its finals.
        phase1(0)
        for b in range(1, BPC):
            phase1(b, interleave=build_phase23_ops(b - 1))
        for f in build_phase23_ops(BPC - 1):
            f()

    nc.finalize()
    return nc


def _get_nc():
    if "nc" not in _cache:
        _cache["nc"] = _build()
    return _cache["nc"]


def _in_maps(wM, wd, e1, e2):
    ident = np.eye(128, dtype=np.float32)
    maps = []
    for i in range(N_CORES):
        sl = slice(i * BPC, (i + 1) * BPC)
        erow = np.ascontiguousarray(
            np.stack([e1[sl], e2[sl]], axis=1).reshape(1, BPC * 2 * D))
        maps.append({
            "wd": np.ascontiguousarray(wd[sl]),
            "wM": np.ascontiguousarray(wM[sl]),
            "erow": erow,
            "ident": ident,
        })
    return maps


def _run(wM, wd, e1, e2, **kw):
    wM = np.asarray(wM, dtype=np.float32)
    wd = np.asarray(wd, dtype=np.float32)
    e1 = np.asarray(e1, dtype=np.float32)
    e2 = np.asarray(e2, dtype=np.float32)
    nc = _get_nc()
    res = run_bass_kernel_spmd(nc, _in_maps(wM, wd, e1, e2), CORE_IDS, **kw)
    out = np.concatenate([r["out"] for r in res.results], axis=0)
    return out, res


def kernel(wM, wd, e1, e2):
    out, _ = _run(wM, wd, e1, e2)
    return out


# revision 39
# speedup vs baseline: 1.1202x; 1.1202x over previous
"""Trainium2 Bass kernel for entity-attention input scaling.

Computes, per batch row b:
    A_k = wd[b] @ e_k[b]          (k = 1, 2)   [S]
    alpha_k = softmax(A_k)
    out[b]  = wM[b] * 0.5 * (alpha_1^2 + alpha_2^2)[:, None]

Sharding: pure data parallel over the batch dim. B=32 batches are split
4-per-core over 8 NeuronCores; no cross-core communication.

Per-core pipeline (per local batch), memory-roofline bound (~50MB DMA/core):
  - wd streamed in 2MB contiguous chunks -> SBUF [128, 4096]
    (s = 2048*c + 16*p + j; p = partition, j in 0..15)
  - logits on the DVE: one fused scalar_tensor_tensor (product + free-axis
    accumulate) per [128, 256] tile against host-broadcast e_k
    -> psA[128, 64] (A_k per (tile, k) col).
  - softmax stats: row max via DVE reduce + PE transpose + DVE reduce;
    exp on ACT with per-partition accumulate, cross-partition sums via a
    ones-vector matmul, 1/Z on DVE reciprocal.  alpha is assembled as
    c_1*E_1^2 + c_2*E_2^2 with c_k = 0.5/Z_k^2 broadcast across partitions
    by a rank-1 matmul (no Ln -> single ACT table load).
  - out = wM * alpha via per-partition scaled multiply, split ACT/DVE.
  - The per-batch stats chain is a long cross-engine dependency chain, so
    emission is software-pipelined at op granularity: batch b's stats and
    finals closures are emitted one per j-tile inside batch b+2's phase 1,
    and the two trailing batches' chains are interleaved with each other,
    so the DVE keeps streaming dot products while stats hop across engines.

Measured on 8 axon-tunneled TRN2 cores: 145.3us NEFF exec, rel err 4.0e-06
(memory roofline for the 50.4MB/core of HBM traffic is ~127-145us).
"""

import numpy as np
from contextlib import ExitStack

import concourse.bacc as bacc
import concourse.tile as tile
from concourse import mybir
from concourse.bass_utils import run_bass_kernel_spmd

B, S, D = 32, 4096, 256
N_CORES = 8
BPC = B // N_CORES          # batches per core
CHUNK = 2048                # S-rows per DMA chunk (2MB)
NCH = S // CHUNK            # chunks per batch
JP = CHUNK // 128           # 128-row tiles per chunk
NT = S // 128               # 128-row tiles per batch
F32 = mybir.dt.float32
AF = mybir.ActivationFunctionType
ALU = mybir.AluOpType
CORE_IDS = list(range(N_CORES))

_cache: dict = {}


def _build():
    nc = bacc.Bacc("TRN2", target_bir_lowering=False, debug=False,
                   num_devices=N_CORES)
    wd_h = nc.declare_dram_parameter("wd", [BPC, S, D], F32, isOutput=False)
    wM_h = nc.declare_dram_parameter("wM", [BPC, S, D], F32, isOutput=False)
    # erow[0, ((b*2 + k)*D + d)] = e_k[b, d]; broadcast on-chip (8KB DMA
    # instead of a 1MB pre-broadcast copy)
    er_h = nc.declare_dram_parameter("erow", [1, BPC * 2 * D], F32,
                                     isOutput=False)
    id_h = nc.declare_dram_parameter("ident", [128, 128], F32, isOutput=False)
    out_h = nc.declare_dram_parameter("out", [BPC, S, D], F32, isOutput=True)

    def chunk_view(h, b, c):
        # [CHUNK, D] contiguous rows -> [128, JP*D]; s = CHUNK*c + JP*p + j
        return h[b, CHUNK * c:CHUNK * (c + 1), :].rearrange(
            "(p j) d -> p (j d)", p=128)

    with tile.TileContext(nc) as tc, ExitStack() as ctx:
        consts = ctx.enter_context(tc.tile_pool(name="consts", bufs=1))
        wd_pool = ctx.enter_context(tc.tile_pool(name="wdp", bufs=3))
        wm_pool = ctx.enter_context(tc.tile_pool(name="wmp", bufs=4))
        out_pool = ctx.enter_context(tc.tile_pool(name="outp", bufs=3))
        scr_pool = ctx.enter_context(tc.tile_pool(name="scrp", bufs=2))
        sm_pool = ctx.enter_context(tc.tile_pool(name="smalls", bufs=2))
        al_pool = ctx.enter_context(tc.tile_pool(name="alphas", bufs=2))
        # two stats chains can be in flight at the kernel tail
        pss_pool = ctx.enter_context(tc.tile_pool(name="pss", bufs=4, space="PSUM"))
        psb_pool = ctx.enter_context(tc.tile_pool(name="psb", bufs=2, space="PSUM"))

        # constants: memset where possible, tiny DMAs on the store ring
        # (idle at kernel start) so nothing delays the first wd chunks.
        onescol = consts.tile([128, 1], F32)
        nc.gpsimd.memset(onescol[:], 1.0)
        onesrow = consts.tile([1, 128], F32)
        nc.gpsimd.memset(onesrow[:], 1.0)
        negone = consts.tile([1, 128], F32)
        nc.gpsimd.memset(negone[:], -1.0)
        ident = consts.tile([128, 128], F32)
        nc.scalar.dma_start(ident[:], id_h[:])
        # e rows: 8KB DMA, then rank-1 matmul broadcast to all partitions
        erow = consts.tile([1, BPC * 2 * D], F32)
        nc.scalar.dma_start(erow[:], er_h[:])
        ebc = consts.tile([128, BPC * 2 * D], F32)
        for q in range(BPC * 2 * D // 512):
            qsl = slice(q * 512, (q + 1) * 512)
            eb_ps = psb_pool.tile([128, 512], F32, tag="ebps")
            nc.tensor.matmul(eb_ps[:], onesrow[:], erow[:, qsl],
                             start=True, stop=True)
            nc.scalar.copy(ebc[:, qsl], eb_ps[:])

        psAs = {}

        def phase1(b, interleave=None):
            # logits: psA[p, 2t+k] = sum_d wd[s(p,t), d] * e_k[d]
            # `interleave`: list of closures (previous batch's stats/finals)
            # emitted one per j-tile so the DVE program keeps streaming dot
            # products while the stats chain hops across engines.
            psA = al_pool.tile([128, 2 * NT], F32, tag="psA")
            psAs[b] = psA
            for c in range(NCH):
                # First chunk of the kernel arrives in 1MB quarters so the
                # DVE starts ~6us earlier; steady state uses one 2MB DMA.
                nparts = 4 if (b == 0 and c == 0) else 1
                jpp = JP // nparts
                wd_ch = wd_pool.tile([128, JP * D], F32, tag="wd")
                full = chunk_view(wd_h, b, c)
                for p_ in range(nparts):
                    fsl = slice(p_ * jpp * D, (p_ + 1) * jpp * D)
                    nc.sync.dma_start(wd_ch[:, fsl], full[:, fsl])
                for j in range(JP):
                    t = c * JP + j
                    wsl = wd_ch[:, j * D:(j + 1) * D]
                    for k in range(2):
                        scr = scr_pool.tile([128, D], F32, tag="scr")
                        nc.vector.scalar_tensor_tensor(
                            scr[:], wsl, 1.0,
                            ebc[:, (b * 2 + k) * D:(b * 2 + k + 1) * D],
                            op0=ALU.mult, op1=ALU.mult,
                            accum_out=psA[:, 2 * t + k:2 * t + k + 1])
                    if interleave:
                        interleave.pop(0)()
            while interleave:
                interleave.pop(0)()

        def build_phase23_ops(b):
            """Batch b's softmax + finals as a list of closures, emitted one
            per j-tile inside the next batch's phase 1 (or directly)."""
            psA = psAs.pop(b)
            st: dict = {}
            ops = []

            def op_mx():
                st["mx"] = sm_pool.tile([128, 1], F32, tag="mx", name="mx")
                nc.vector.tensor_reduce(st["mx"][:], psA[:],
                                        axis=mybir.AxisListType.X, op=ALU.max)

            def op_tmax():
                st["tmax"] = pss_pool.tile([1, 128], F32, tag="pssm", name="tmax")
                nc.tensor.transpose(st["tmax"][:], st["mx"][:], ident[:])

            def op_m2():
                st["m2"] = sm_pool.tile([1, 1], F32, tag="m2", name="m2")
                nc.vector.tensor_reduce(st["m2"][:], st["tmax"][:],
                                        axis=mybir.AxisListType.X, op=ALU.max)

            def op_mneg_mm():
                st["mneg_ps"] = pss_pool.tile([128, 1], F32, tag="pssm", name="mneg_ps")
                nc.tensor.matmul(st["mneg_ps"][:], negone[:], st["m2"][:],
                                 start=True, stop=True)

            def op_mneg_cp():
                st["mneg"] = sm_pool.tile([128, 1], F32, tag="mneg", name="mneg")
                nc.scalar.copy(st["mneg"][:], st["mneg_ps"][:])

            def op_exp(k):
                psA_v = psA[:].rearrange("p (t k) -> p k t", k=2)
                if "E" not in st:
                    st["E"] = al_pool.tile([128, 2 * NT], F32, tag="E", name="E")
                    st["s12"] = sm_pool.tile([128, 2], F32, tag="s12", name="s12")
                E_v = st["E"][:].rearrange("p (t k) -> p k t", k=2)
                nc.scalar.activation(E_v[:, k, :], psA_v[:, k, :], AF.Exp,
                                     bias=st["mneg"][:], scale=1.0,
                                     accum_out=st["s12"][:, k:k + 1])

            def op_zsum():
                st["zsum"] = pss_pool.tile([1, 2], F32, tag="pssm", name="zsum")
                nc.tensor.matmul(st["zsum"][:], onescol[:], st["s12"][:],
                                 start=True, stop=True)

            def op_zinv():
                st["zinv"] = sm_pool.tile([1, 2], F32, tag="zinv", name="zinv")
                nc.vector.reciprocal(st["zinv"][:], st["zsum"][:])
                st["zz"] = sm_pool.tile([1, 2], F32, tag="zz", name="zz")
                nc.vector.tensor_scalar(st["zz"][:], st["zinv"][:], 0.5, None,
                                        op0=ALU.mult)
                nc.vector.tensor_mul(st["zz"][:], st["zz"][:], st["zinv"][:])

            def op_cps():
                st["c_ps"] = pss_pool.tile([128, 2], F32, tag="pssm", name="c_ps")
                nc.tensor.matmul(st["c_ps"][:], onesrow[:], st["zz"][:],
                                 start=True, stop=True)

            def op_c12():
                st["c12"] = sm_pool.tile([128, 2], F32, tag="c12", name="c12")
                nc.scalar.copy(st["c12"][:], st["c_ps"][:])

            def op_esq():
                st["esq"] = al_pool.tile([128, 2 * NT], F32, tag="esq", name="esq")
                nc.vector.tensor_mul(st["esq"][:], st["E"][:], st["E"][:])

            def op_alpha():
                esq_v = st["esq"][:].rearrange("p (t k) -> p k t", k=2)
                atmp = al_pool.tile([128, NT], F32, tag="atmp")
                nc.vector.tensor_scalar_mul(atmp[:], esq_v[:, 1, :],
                                            st["c12"][:, 1:2])
                st["alpha"] = al_pool.tile([128, NT], F32, tag="alpha", name="alpha")
                nc.vector.scalar_tensor_tensor(st["alpha"][:], esq_v[:, 0, :],
                                               st["c12"][:, 0:1], atmp[:],
                                               op0=ALU.mult, op1=ALU.add)

            ops += [op_mx, op_tmax, op_m2, op_mneg_mm, op_mneg_cp,
                    lambda: op_exp(0), lambda: op_exp(1),
                    op_zsum, op_zinv, op_cps, op_c12, op_esq, op_alpha]

            # ---- out = wM * alpha ----
            # Last batch: 1MB quarters + all finals on the DVE so loads,
            # finals and stores pipeline tightly at the kernel tail.
            nparts = 4 if b == BPC - 1 else 1
            jpp = JP // nparts
            tail_ops = []

            def fin_part(c, p_, jpp_, eng, load):
                # one wM sub-load (optional) + its finals + its 1MB+ store
                def op(c=c, p_=p_, jpp_=jpp_, eng=eng, load=load):
                    wm_ch = st[("wm", c)]
                    out_ch = st[("out", c)]
                    fsl = slice(p_ * jpp_ * D, (p_ + 1) * jpp_ * D)
                    if load:
                        nc.sync.dma_start(wm_ch[:, fsl],
                                          chunk_view(wM_h, b, c)[:, fsl])
                    for j in range(p_ * jpp_, (p_ + 1) * jpp_):
                        t = c * JP + j
                        sl = slice(j * D, (j + 1) * D)
                        if eng is nc.vector:
                            nc.vector.tensor_scalar_mul(
                                out_ch[:, sl], wm_ch[:, sl],
                                st["alpha"][:, t:t + 1])
                        else:
                            nc.scalar.mul(out_ch[:, sl], wm_ch[:, sl],
                                          st["alpha"][:, t:t + 1])
                    nc.scalar.dma_start(
                        chunk_view(out_h, b, c)[:, fsl], out_ch[:, fsl])
                return op

            for c in range(NCH):
                def op_wm_alloc(b=b, c=c):
                    st[("wm", c)] = wm_pool.tile([128, JP * D], F32, tag="wm", name="wm")
                    st[("out", c)] = out_pool.tile([128, JP * D], F32, tag="out", name="out")
                ops.append(op_wm_alloc)
                if b == BPC - 2:
                    # Second-to-last batch: first half of each chunk on ACT
                    # during the dot stream (ACT has slack there); second
                    # half deferred to the DVE *after* the stream ends (via
                    # the tail pad below), so ACT's in-order queue is clear
                    # when the last batch's stats hops (exp is ACT-only)
                    # arrive.  Each half stores its own contiguous 1MB as
                    # soon as it finishes.
                    def op_wm_load2(b=b, c=c):
                        nc.sync.dma_start(st[("wm", c)][:],
                                          chunk_view(wM_h, b, c)[:])
                    ops.append(op_wm_load2)
                    ops.append(fin_part(c, 0, JP // 2, nc.scalar, False))
                    tail_ops.append(fin_part(c, 1, JP // 2, nc.vector, False))
                elif b == BPC - 1:
                    # Last batch: 1MB quarters, finals on the then-idle DVE.
                    for p_ in range(nparts):
                        ops.append(fin_part(c, p_, jpp, nc.vector, True))
                else:
                    ops.append(fin_part(c, 0, JP, nc.scalar, True))
            if tail_ops:
                # pad so the deferred closures pop only in phase1's trailing
                # while-loop, i.e. after every dot product is emitted
                # (phase1 pops one closure per j-tile; NCH*JP slots).
                ops += [lambda: None] * max(0, NCH * JP - len(ops))
                ops += tail_ops
            return ops

        # software pipeline: batch b's stats/finals closures are emitted one
        # per j-tile inside batch b+1's phase 1, so batches 0..2 fully drain
        # (stats on their engines, finals on ACT, stores) while the DVE
        # streams dot products; only batch 3's chain remains in the tail,
        # where it gets the then-idle DVE for its finals.
        phase1(0)
        for b in range(1, BPC):
            phase1(b, interleave=build_phase23_ops(b - 1))
        for f in build_phase23_ops(BPC - 1):
            f()

    nc.finalize()
    return nc


def _get_nc():
    if "nc" not in _cache:
        _cache["nc"] = _build()
    return _cache["nc"]


def _in_maps(wM, wd, e1, e2):
    ident = np.eye(128, dtype=np.float32)
    maps = []
    for i in range(N_CORES):
        sl = slice(i * BPC, (i + 1) * BPC)
        erow = np.ascontiguousarray(
            np.stack([e1[sl], e2[sl]], axis=1).reshape(1, BPC * 2 * D))
        maps.append({
            "wd": np.ascontiguousarray(wd[sl]),
            "wM": np.ascontiguousarray(wM[sl]),
            "erow": erow,
            "ident": ident,
        })
    return maps


def _run(wM, wd, e1, e2, **kw):
    wM = np.asarray(wM, dtype=np.float32)
    wd = np.asarray(wd, dtype=np.float32)
    e1 = np.asarray(e1, dtype=np.float32)
    e2 = np.asarray(e2, dtype=np.float32)
    nc = _get_nc()
    res = run_bass_kernel_spmd(nc, _in_maps(wM, wd, e1, e2), CORE_IDS, **kw)
    out = np.concatenate([r["out"] for r in res.results], axis=0)
    return out, res


def kernel(wM, wd, e1, e2):
    out, _ = _run(wM, wd, e1, e2)
    return out


# revision 40
# speedup vs baseline: 1.1215x; 1.0012x over previous
"""Trainium2 Bass kernel for entity-attention input scaling.

Computes, per batch row b:
    A_k = wd[b] @ e_k[b]          (k = 1, 2)   [S]
    alpha_k = softmax(A_k)
    out[b]  = wM[b] * 0.5 * (alpha_1^2 + alpha_2^2)[:, None]

Sharding: pure data parallel over the batch dim. B=32 batches are split
4-per-core over 8 NeuronCores; no cross-core communication.

Per-core pipeline (per local batch), memory-roofline bound (~50MB DMA/core):
  - wd streamed in 2MB contiguous chunks -> SBUF [128, 4096]
    (s = 2048*c + 16*p + j; p = partition, j in 0..15)
  - logits on the DVE: one fused scalar_tensor_tensor (product + free-axis
    accumulate) per [128, 256] tile against host-broadcast e_k
    -> psA[128, 64] (A_k per (tile, k) col).
  - softmax stats: row max via DVE reduce + PE transpose + DVE reduce;
    exp on ACT with per-partition accumulate, cross-partition sums via a
    ones-vector matmul, 1/Z on DVE reciprocal.  alpha is assembled as
    c_1*E_1^2 + c_2*E_2^2 with c_k = 0.5/Z_k^2 broadcast across partitions
    by a rank-1 matmul (no Ln -> single ACT table load).
  - out = wM * alpha via per-partition scaled multiply, split ACT/DVE.
  - The per-batch stats chain is a long cross-engine dependency chain, so
    emission is software-pipelined at op granularity: batch b's stats and
    finals closures are emitted one per j-tile inside batch b+2's phase 1,
    and the two trailing batches' chains are interleaved with each other,
    so the DVE keeps streaming dot products while stats hop across engines.

Measured on 8 axon-tunneled TRN2 cores: 139.3-145.3us NEFF exec across
runs (run-to-run variance ~±3-6us; best sample 139,281ns), rel err 4.0e-06
(memory roofline for the 50.4MB/core of HBM traffic is ~127-145us).
"""

import numpy as np
from contextlib import ExitStack

import concourse.bacc as bacc
import concourse.tile as tile
from concourse import mybir
from concourse.bass_utils import run_bass_kernel_spmd

B, S, D = 32, 4096, 256
N_CORES = 8
BPC = B // N_CORES          # batches per core
CHUNK = 2048                # S-rows per DMA chunk (2MB)
NCH = S // CHUNK            # chunks per batch
JP = CHUNK // 128           # 128-row tiles per chunk
NT = S // 128               # 128-row tiles per batch
F32 = mybir.dt.float32
AF = mybir.ActivationFunctionType
ALU = mybir.AluOpType
CORE_IDS = list(range(N_CORES))

_cache: dict = {}


def _build():
    nc = bacc.Bacc("TRN2", target_bir_lowering=False, debug=False,
                   num_devices=N_CORES)
    wd_h = nc.declare_dram_parameter("wd", [BPC, S, D], F32, isOutput=False)
    wM_h = nc.declare_dram_parameter("wM", [BPC, S, D], F32, isOutput=False)
    # erow[0, ((b*2 + k)*D + d)] = e_k[b, d]; broadcast on-chip (8KB DMA
    # instead of a 1MB pre-broadcast copy)
    er_h = nc.declare_dram_parameter("erow", [1, BPC * 2 * D], F32,
                                     isOutput=False)
    id_h = nc.declare_dram_parameter("ident", [128, 128], F32, isOutput=False)
    out_h = nc.declare_dram_parameter("out", [BPC, S, D], F32, isOutput=True)

    def chunk_view(h, b, c):
        # [CHUNK, D] contiguous rows -> [128, JP*D]; s = CHUNK*c + JP*p + j
        return h[b, CHUNK * c:CHUNK * (c + 1), :].rearrange(
            "(p j) d -> p (j d)", p=128)

    with tile.TileContext(nc) as tc, ExitStack() as ctx:
        consts = ctx.enter_context(tc.tile_pool(name="consts", bufs=1))
        wd_pool = ctx.enter_context(tc.tile_pool(name="wdp", bufs=3))
        wm_pool = ctx.enter_context(tc.tile_pool(name="wmp", bufs=4))
        out_pool = ctx.enter_context(tc.tile_pool(name="outp", bufs=3))
        scr_pool = ctx.enter_context(tc.tile_pool(name="scrp", bufs=2))
        sm_pool = ctx.enter_context(tc.tile_pool(name="smalls", bufs=2))
        al_pool = ctx.enter_context(tc.tile_pool(name="alphas", bufs=2))
        # two stats chains can be in flight at the kernel tail
        pss_pool = ctx.enter_context(tc.tile_pool(name="pss", bufs=4, space="PSUM"))
        psb_pool = ctx.enter_context(tc.tile_pool(name="psb", bufs=2, space="PSUM"))

        # constants: memset where possible, tiny DMAs on the store ring
        # (idle at kernel start) so nothing delays the first wd chunks.
        onescol = consts.tile([128, 1], F32)
        nc.gpsimd.memset(onescol[:], 1.0)
        onesrow = consts.tile([1, 128], F32)
        nc.gpsimd.memset(onesrow[:], 1.0)
        negone = consts.tile([1, 128], F32)
        nc.gpsimd.memset(negone[:], -1.0)
        ident = consts.tile([128, 128], F32)
        nc.scalar.dma_start(ident[:], id_h[:])
        # e rows: 8KB DMA, then rank-1 matmul broadcast to all partitions
        erow = consts.tile([1, BPC * 2 * D], F32)
        nc.scalar.dma_start(erow[:], er_h[:])
        ebc = consts.tile([128, BPC * 2 * D], F32)
        for q in range(BPC * 2 * D // 512):
            qsl = slice(q * 512, (q + 1) * 512)
            eb_ps = psb_pool.tile([128, 512], F32, tag="ebps")
            nc.tensor.matmul(eb_ps[:], onesrow[:], erow[:, qsl],
                             start=True, stop=True)
            nc.scalar.copy(ebc[:, qsl], eb_ps[:])

        psAs = {}

        def phase1(b, interleave=None):
            # logits: psA[p, 2t+k] = sum_d wd[s(p,t), d] * e_k[d]
            # `interleave`: list of closures (previous batch's stats/finals)
            # emitted one per j-tile so the DVE program keeps streaming dot
            # products while the stats chain hops across engines.
            psA = al_pool.tile([128, 2 * NT], F32, tag="psA")
            psAs[b] = psA
            for c in range(NCH):
                # First chunk of the kernel arrives in 1MB quarters so the
                # DVE starts ~6us earlier; steady state uses one 2MB DMA.
                nparts = 4 if (b == 0 and c == 0) else 1
                jpp = JP // nparts
                wd_ch = wd_pool.tile([128, JP * D], F32, tag="wd")
                full = chunk_view(wd_h, b, c)
                for p_ in range(nparts):
                    fsl = slice(p_ * jpp * D, (p_ + 1) * jpp * D)
                    nc.sync.dma_start(wd_ch[:, fsl], full[:, fsl])
                for j in range(JP):
                    t = c * JP + j
                    wsl = wd_ch[:, j * D:(j + 1) * D]
                    for k in range(2):
                        scr = scr_pool.tile([128, D], F32, tag="scr")
                        nc.vector.scalar_tensor_tensor(
                            scr[:], wsl, 1.0,
                            ebc[:, (b * 2 + k) * D:(b * 2 + k + 1) * D],
                            op0=ALU.mult, op1=ALU.mult,
                            accum_out=psA[:, 2 * t + k:2 * t + k + 1])
                    if interleave:
                        interleave.pop(0)()
            while interleave:
                interleave.pop(0)()

        def build_phase23_ops(b):
            """Batch b's softmax + finals as a list of closures, emitted one
            per j-tile inside the next batch's phase 1 (or directly)."""
            psA = psAs.pop(b)
            st: dict = {}
            ops = []

            def op_mx():
                st["mx"] = sm_pool.tile([128, 1], F32, tag="mx", name="mx")
                nc.vector.tensor_reduce(st["mx"][:], psA[:],
                                        axis=mybir.AxisListType.X, op=ALU.max)

            def op_tmax():
                st["tmax"] = pss_pool.tile([1, 128], F32, tag="pssm", name="tmax")
                nc.tensor.transpose(st["tmax"][:], st["mx"][:], ident[:])

            def op_m2():
                st["m2"] = sm_pool.tile([1, 1], F32, tag="m2", name="m2")
                nc.vector.tensor_reduce(st["m2"][:], st["tmax"][:],
                                        axis=mybir.AxisListType.X, op=ALU.max)

            def op_mneg_mm():
                st["mneg_ps"] = pss_pool.tile([128, 1], F32, tag="pssm", name="mneg_ps")
                nc.tensor.matmul(st["mneg_ps"][:], negone[:], st["m2"][:],
                                 start=True, stop=True)

            def op_mneg_cp():
                st["mneg"] = sm_pool.tile([128, 1], F32, tag="mneg", name="mneg")
                nc.scalar.copy(st["mneg"][:], st["mneg_ps"][:])

            def op_exp(k):
                psA_v = psA[:].rearrange("p (t k) -> p k t", k=2)
                if "E" not in st:
                    st["E"] = al_pool.tile([128, 2 * NT], F32, tag="E", name="E")
                    st["s12"] = sm_pool.tile([128, 2], F32, tag="s12", name="s12")
                E_v = st["E"][:].rearrange("p (t k) -> p k t", k=2)
                nc.scalar.activation(E_v[:, k, :], psA_v[:, k, :], AF.Exp,
                                     bias=st["mneg"][:], scale=1.0,
                                     accum_out=st["s12"][:, k:k + 1])

            def op_zsum():
                st["zsum"] = pss_pool.tile([1, 2], F32, tag="pssm", name="zsum")
                nc.tensor.matmul(st["zsum"][:], onescol[:], st["s12"][:],
                                 start=True, stop=True)

            def op_zinv():
                st["zinv"] = sm_pool.tile([1, 2], F32, tag="zinv", name="zinv")
                nc.vector.reciprocal(st["zinv"][:], st["zsum"][:])
                st["zz"] = sm_pool.tile([1, 2], F32, tag="zz", name="zz")
                nc.vector.tensor_scalar(st["zz"][:], st["zinv"][:], 0.5, None,
                                        op0=ALU.mult)
                nc.vector.tensor_mul(st["zz"][:], st["zz"][:], st["zinv"][:])

            def op_cps():
                st["c_ps"] = pss_pool.tile([128, 2], F32, tag="pssm", name="c_ps")
                nc.tensor.matmul(st["c_ps"][:], onesrow[:], st["zz"][:],
                                 start=True, stop=True)

            def op_c12():
                st["c12"] = sm_pool.tile([128, 2], F32, tag="c12", name="c12")
                nc.scalar.copy(st["c12"][:], st["c_ps"][:])

            def op_esq():
                st["esq"] = al_pool.tile([128, 2 * NT], F32, tag="esq", name="esq")
                nc.vector.tensor_mul(st["esq"][:], st["E"][:], st["E"][:])

            def op_alpha():
                esq_v = st["esq"][:].rearrange("p (t k) -> p k t", k=2)
                atmp = al_pool.tile([128, NT], F32, tag="atmp")
                nc.vector.tensor_scalar_mul(atmp[:], esq_v[:, 1, :],
                                            st["c12"][:, 1:2])
                st["alpha"] = al_pool.tile([128, NT], F32, tag="alpha", name="alpha")
                nc.vector.scalar_tensor_tensor(st["alpha"][:], esq_v[:, 0, :],
                                               st["c12"][:, 0:1], atmp[:],
                                               op0=ALU.mult, op1=ALU.add)

            ops += [op_mx, op_tmax, op_m2, op_mneg_mm, op_mneg_cp,
                    lambda: op_exp(0), lambda: op_exp(1),
                    op_zsum, op_zinv, op_cps, op_c12, op_esq, op_alpha]

            # ---- out = wM * alpha ----
            # Last batch: 1MB quarters + all finals on the DVE so loads,
            # finals and stores pipeline tightly at the kernel tail.
            nparts = 4 if b == BPC - 1 else 1
            jpp = JP // nparts
            tail_ops = []

            def fin_part(c, p_, jpp_, eng, load):
                # one wM sub-load (optional) + its finals + its 1MB+ store
                def op(c=c, p_=p_, jpp_=jpp_, eng=eng, load=load):
                    wm_ch = st[("wm", c)]
                    out_ch = st[("out", c)]
                    fsl = slice(p_ * jpp_ * D, (p_ + 1) * jpp_ * D)
                    if load:
                        nc.sync.dma_start(wm_ch[:, fsl],
                                          chunk_view(wM_h, b, c)[:, fsl])
                    for j in range(p_ * jpp_, (p_ + 1) * jpp_):
                        t = c * JP + j
                        sl = slice(j * D, (j + 1) * D)
                        if eng is nc.vector:
                            nc.vector.tensor_scalar_mul(
                                out_ch[:, sl], wm_ch[:, sl],
                                st["alpha"][:, t:t + 1])
                        else:
                            nc.scalar.mul(out_ch[:, sl], wm_ch[:, sl],
                                          st["alpha"][:, t:t + 1])
                    nc.scalar.dma_start(
                        chunk_view(out_h, b, c)[:, fsl], out_ch[:, fsl])
                return op

            for c in range(NCH):
                def op_wm_alloc(b=b, c=c):
                    st[("wm", c)] = wm_pool.tile([128, JP * D], F32, tag="wm", name="wm")
                    st[("out", c)] = out_pool.tile([128, JP * D], F32, tag="out", name="out")
                ops.append(op_wm_alloc)
                if b == BPC - 2:
                    # Second-to-last batch: first half of each chunk on ACT
                    # during the dot stream (ACT has slack there); second
                    # half deferred to the DVE *after* the stream ends (via
                    # the tail pad below), so ACT's in-order queue is clear
                    # when the last batch's stats hops (exp is ACT-only)
                    # arrive.  Each half stores its own contiguous 1MB as
                    # soon as it finishes.
                    def op_wm_load2(b=b, c=c):
                        nc.sync.dma_start(st[("wm", c)][:],
                                          chunk_view(wM_h, b, c)[:])
                    ops.append(op_wm_load2)
                    ops.append(fin_part(c, 0, JP // 2, nc.scalar, False))
                    tail_ops.append(fin_part(c, 1, JP // 2, nc.vector, False))
                elif b == BPC - 1:
                    # Last batch: 1MB quarters, finals on the then-idle DVE.
                    for p_ in range(nparts):
                        ops.append(fin_part(c, p_, jpp, nc.vector, True))
                else:
                    ops.append(fin_part(c, 0, JP, nc.scalar, True))
            if tail_ops:
                # pad so the deferred closures pop only in phase1's trailing
                # while-loop, i.e. after every dot product is emitted
                # (phase1 pops one closure per j-tile; NCH*JP slots).
                ops += [lambda: None] * max(0, NCH * JP - len(ops))
                ops += tail_ops
            return ops

        # software pipeline: batch b's stats/finals closures are emitted one
        # per j-tile inside batch b+1's phase 1, so batches 0..2 fully drain
        # (stats on their engines, finals on ACT, stores) while the DVE
        # streams dot products; only batch 3's chain remains in the tail,
        # where it gets the then-idle DVE for its finals.
        phase1(0)
        for b in range(1, BPC):
            phase1(b, interleave=build_phase23_ops(b - 1))
        for f in build_phase23_ops(BPC - 1):
            f()

    nc.finalize()
    return nc


def _get_nc():
    if "nc" not in _cache:
        _cache["nc"] = _build()
    return _cache["nc"]


def _in_maps(wM, wd, e1, e2):
    ident = np.eye(128, dtype=np.float32)
    maps = []
    for i in range(N_CORES):
        sl = slice(i * BPC, (i + 1) * BPC)
        erow = np.ascontiguousarray(
            np.stack([e1[sl], e2[sl]], axis=1).reshape(1, BPC * 2 * D))
        maps.append({
            "wd": np.ascontiguousarray(wd[sl]),
            "wM": np.ascontiguousarray(wM[sl]),
            "erow": erow,
            "ident": ident,
        })
    return maps


def _run(wM, wd, e1, e2, **kw):
    wM = np.asarray(wM, dtype=np.float32)
    wd = np.asarray(wd, dtype=np.float32)
    e1 = np.asarray(e1, dtype=np.float32)
    e2 = np.asarray(e2, dtype=np.float32)
    nc = _get_nc()
    res = run_bass_kernel_spmd(nc, _in_maps(wM, wd, e1, e2), CORE_IDS, **kw)
    out = np.concatenate([r["out"] for r in res.results], axis=0)
    return out, res


def kernel(wM, wd, e1, e2):
    out, _ = _run(wM, wd, e1, e2)
    return out
